# revision 15
# baseline (speedup 1.0000x reference)
"""MI-LSTM (attention LSTM) + LSTM + linear head for Trainium2, 8-core batch-parallel.

Model (per timestep, per batch row b):
  gm = y@W_main + h@U_main + b_main -> i,f,o,cm gates
  ga[k] = x_k@W_aux[k] + h@U_aux[k] + b_aux[k] -> i_k (sigmoid), c_k (tanh)
  candidates l = [i*cm, i_k*c_k] (9, H)
  u_k = tanh(l_k . (W_att @ c) + b_att); a = softmax(u); L = sum a_k l_k
  c' = f*c + L; h' = o*tanh(c')
Then a standard LSTM over the h-sequence, then relu + linear to scalar.

Mapping: batch sharded 8 ways (256 rows/core = 2 partition tiles of 128).
Batch-major layout (batch on partitions). Gate matmuls use a stacked
feature-major input tile XYHT (110 rows: 5 y + 40 x + 64 h + 1 ones-for-bias),
rebuilt each step via PE transposes. exp() for softmax is computed as
(1+t)/(1-t) with t=tanh(u/2) so only one ACT table set is ever loaded.
"""

import os
import numpy as np
import ml_dtypes

import concourse.bacc as bacc
import concourse.bass as bass
import concourse.mybir as mybir
from concourse.tile import TileContext
from concourse.bass_utils import run_bass_kernel_spmd

F32 = mybir.dt.float32
BF16 = mybir.dt.bfloat16
ALU = mybir.AluOpType
ACTF = mybir.ActivationFunctionType
AX = mybir.AxisListType

S, B, F, H, K = 256, 2048, 5, 64, 8
NC = 8
BL = B // NC          # 256 batch rows per core
NT = BL // 128        # 2 partition tiles
KIN = 128   # stacked rows: 0:5 y, 5:45 x, 45 bias-ones, 46:64 zero, 64:128 h
NCAND = K + 1         # 9 candidates

LAST_RESULTS = {}


def _build(n_steps: int, b_att: float):
    nc = bacc.Bacc(None, target_bir_lowering=False)

    xin = nc.dram_tensor("xin", [n_steps, 1 + K, BL, F], F32, kind="ExternalInput")
    wall = nc.dram_tensor("wall", [KIN, 1280], BF16, kind="ExternalInput")
    watt = nc.dram_tensor("watt", [H, H], BF16, kind="ExternalInput")
    wc_a = nc.dram_tensor("wc_a", [H, 4 * H], BF16, kind="ExternalInput")
    wc_b = nc.dram_tensor("wc_b", [H, 4 * H], BF16, kind="ExternalInput")
    bias2 = nc.dram_tensor("bias2", [1, 4 * H], BF16, kind="ExternalInput")
    linw = nc.dram_tensor("linw", [128, H], BF16, kind="ExternalInput")
    ones1 = nc.dram_tensor("ones1", [1, 128], BF16, kind="ExternalInput")
    idf32 = nc.dram_tensor("idf32", [128, 128], F32, kind="ExternalInput")
    onesrow = nc.dram_tensor("onesrow", [1, BL], BF16, kind="ExternalInput")
    out = nc.dram_tensor("out", [n_steps, BL, 1], F32, kind="ExternalOutput")

    with TileContext(nc) as tc:
        with (
            tc.tile_pool(name="state", bufs=1) as st,
            tc.tile_pool(name="wts", bufs=1) as wp,
            tc.tile_pool(name="work", bufs=2) as wk,
            tc.tile_pool(name="raw", bufs=3) as rawp,
            tc.tile_pool(name="gpsum", bufs=1, space="PSUM") as gp,
            tc.tile_pool(name="mpsum", bufs=1, space="PSUM") as mp,
            tc.tile_pool(name="vpsum", bufs=1, space="PSUM") as vp,
            tc.tile_pool(name="xpsum", bufs=1, space="PSUM") as xp,
        ):
            # ---- persistent state / weights in SBUF ----
            W = wp.tile([KIN, 1280], BF16, tag="wall")
            WA = wp.tile([H, H], BF16, tag="watt")
            WCA = wp.tile([H, 4 * H], BF16, tag="wc_a")
            WCB = wp.tile([H, 4 * H], BF16, tag="wc_b")
            B2 = wp.tile([1, 4 * H], BF16, tag="bias2")
            LW = wp.tile([128, H], BF16, tag="linw")
            ON1 = wp.tile([1, 128], BF16, tag="ones1")
            IDF = wp.tile([128, 128], F32, tag="idf32")
            for t_, d_ in ((W, wall), (WA, watt), (WCA, wc_a), (WCB, wc_b),
                           (B2, bias2), (LW, linw), (ON1, ones1),
                           (IDF, idf32)):
                nc.sync.dma_start(t_[:], d_[:])

            XYHT = st.tile([KIN, BL], BF16, tag="xyht")      # stacked gate-matmul input
            CT = st.tile([H, BL], BF16, tag="ct")            # c^T for the v matmul
            HC1 = st.tile([128, 2 * 128], F32, tag="hc1")    # [h|c] per tile
            HC2 = st.tile([128, 2 * 128], F32, tag="hc2")    # phase-2 [h|c]
            H2T = st.tile([H, BL], BF16, tag="h2t")
            HST = st.tile([H, n_steps * BL], BF16, tag="hst")  # h1^T history
            OACC = st.tile([128, NT * n_steps], F32, tag="oacc")

            nc.vector.memset(XYHT[32:64, :], 0.0)
            nc.vector.memset(XYHT[64:128, :], 0.0)
            nc.sync.dma_start(XYHT[45:46, :], onesrow[:])
            nc.vector.memset(CT[:], 0.0)
            nc.vector.memset(HC1[:], 0.0)
            nc.vector.memset(HC2[:], 0.0)
            nc.vector.memset(H2T[:], 0.0)

            xv = xin.rearrange("s i (tau p) f -> s p tau i f", p=128)

            # ================= phase 1: MI-LSTM =================
            for t in range(n_steps):
                # transpose [h|c] (fp32) -> misc psum cols 0:256
                misc = mp.tile([128, 512], F32, tag="misc")
                for tau in range(NT):
                    nc.tensor.transpose(
                        misc[0:128, tau * 128:(tau + 1) * 128],
                        HC1[:, tau * 128:(tau + 1) * 128], IDF[:])
                # h^T -> XYHT rows 45:109 ; c^T -> CT   (cast to bf16)
                nc.vector.tensor_copy(XYHT[64:128, :], misc[0:64, 0:256])
                nc.vector.tensor_copy(CT[:], misc[64:128, 0:256])
                if t > 0:
                    nc.vector.tensor_copy(
                        HST[:, (t - 1) * BL:t * BL], XYHT[64:128, :])

                # x/y slab for step t: load (128, 2*9*5) bf16, transpose to
                # feature-major (45, 256) in misc cols 256:512
                raw = rawp.tile([128, NT * 45], F32, tag="raw")
                for tau in range(NT):
                    nc.sync.dma_start(raw[:, tau * 45:(tau + 1) * 45],
                                      xv[t, :, tau])
                xtp = xp.tile([45, 256], F32, tag="xt")
                for tau in range(NT):
                    nc.tensor.transpose(
                        xtp[0:45, tau * 128:(tau + 1) * 128],
                        raw[:, tau * 45:(tau + 1) * 45], IDF[:])
                nc.vector.tensor_copy(XYHT[0:45, :], xtp[0:45, 0:256])

                # gate matmuls: psum cols [sigA t0|t1, tanA t0|t1, main t0|t1]
                gps = gp.tile([128, 2560], F32, tag="gates")
                for tau in range(NT):
                    lhsT = XYHT[:, tau * 128:(tau + 1) * 128]
                    nc.tensor.matmul(gps[:, tau * 512:(tau + 1) * 512],
                                     lhsT, W[:, 0:512], start=True, stop=True)
                    nc.tensor.matmul(gps[:, 1024 + tau * 512:1024 + (tau + 1) * 512],
                                     lhsT, W[:, 512:1024], start=True, stop=True)
                    nc.tensor.matmul(gps[:, 2048 + tau * 256:2048 + (tau + 1) * 256],
                                     lhsT, W[:, 1024:1280], start=True, stop=True)

                # v = c @ W_att^T  (batch-major), both tiles into one psum bank
                vps = vp.tile([128, 128], F32, tag="v")
                for tau in range(NT):
                    nc.tensor.matmul(vps[:, tau * 64:(tau + 1) * 64],
                                     CT[:, tau * 128:(tau + 1) * 128], WA[:],
                                     start=True, stop=True)

                # activations (PSUM -> SBUF bf16)
                sig = wk.tile([128, NT * 576], BF16, tag="sig")
                tau_t = wk.tile([128, NT * 576], BF16, tag="tau")
                fo = wk.tile([128, NT * 128], BF16, tag="fo")
                sigA = gps[:, 0:1024].rearrange("p (t c) -> p t c", t=2)
                tanA = gps[:, 1024:2048].rearrange("p (t c) -> p t c", t=2)
                mn = gps[:, 2048:2560].rearrange("p (t c) -> p t c", t=2)
                sigv = sig[:].rearrange("p (t c) -> p t c", t=2)
                tauv = tau_t[:].rearrange("p (t c) -> p t c", t=2)
                nc.scalar.activation(sigv[:, :, 64:576], sigA, ACTF.Sigmoid)
                nc.scalar.activation(tauv[:, :, 64:576], tanA, ACTF.Tanh)
                nc.scalar.activation(sigv[:, :, 0:64], mn[:, :, 0:64], ACTF.Sigmoid)
                nc.scalar.activation(
                    fo[:].rearrange("p (t c) -> p t c", t=2),
                    mn[:, :, 64:192], ACTF.Sigmoid)
                nc.scalar.activation(tauv[:, :, 0:64], mn[:, :, 192:256], ACTF.Tanh)

                # candidates and attention
                l_t = wk.tile([128, NT * 576], BF16, tag="l")
                nc.vector.tensor_mul(l_t[:], sig[:], tau_t[:])
                vrep = wk.tile([128, NT * 576], BF16, tag="vrep")
                nc.scalar.activation(
                    vrep[:].rearrange("p (t k h) -> p t k h", k=NCAND, h=H),
                    (vps[:].rearrange("p (t h) -> p t h", t=2)
                     .unsqueeze(2).broadcast_to((128, 2, NCAND, H))),
                    ACTF.Identity)
                z_t = wk.tile([128, NT * 576], BF16, tag="z")
                nc.vector.tensor_mul(z_t[:], l_t[:], vrep[:])
                u_t = wk.tile([128, NT * NCAND], F32, tag="u")
                nc.vector.tensor_reduce(
                    u_t[:],
                    z_t[:].rearrange("p (t k h) -> p t k h", k=NCAND, h=H),
                    AX.X, ALU.add)
                # softmax via exp(u) = (1+t2)/(1-t2), t2 = tanh(u/2)
                ut2 = wk.tile([128, NT * NCAND], F32, tag="ut2")
                nc.scalar.activation(ut2[:], u_t[:], ACTF.Tanh, bias=b_att, scale=1.0)
                t2 = wk.tile([128, NT * NCAND], F32, tag="t2")
                nc.scalar.activation(t2[:], ut2[:], ACTF.Tanh, scale=0.5)
                q_t = wk.tile([128, NT * NCAND], F32, tag="q")
                nc.vector.tensor_scalar(q_t[:], t2[:], -1.0, 1.0, ALU.mult, ALU.add)
                rq = wk.tile([128, NT * NCAND], F32, tag="rq")
                nc.vector.reciprocal_approx_fast(rq[:], q_t[:])
                r_t = wk.tile([128, NT * NCAND], BF16, tag="r")
                nc.vector.scalar_tensor_tensor(
                    r_t[:], t2[:], 1.0, rq[:], ALU.add, ALU.mult)
                s_t = wk.tile([128, NT], F32, tag="s")
                nc.vector.tensor_reduce(
                    s_t[:],
                    r_t[:].rearrange("p (t k) -> p t k", t=2), AX.X, ALU.add)
                rs = wk.tile([128, NT], F32, tag="rs")
                nc.vector.reciprocal_approx_fast(rs[:], s_t[:])

                aw = wk.tile([128, NT * 576], BF16, tag="aw")
                rb = (r_t[:].rearrange("p (t k) -> p t k", t=2)
                      .unsqueeze(3).broadcast_to((128, 2, NCAND, H)))
                nc.vector.tensor_tensor(
                    aw[:].rearrange("p (t k h) -> p t k h", k=NCAND, h=H),
                    l_t[:].rearrange("p (t k h) -> p t k h", k=NCAND, h=H),
                    rb, ALU.mult)
                Lp = wk.tile([128, NT * H], F32, tag="L")
                nc.vector.tensor_reduce(
                    Lp[:],
                    aw[:].rearrange("p (t k h) -> p t h k", k=NCAND, h=H),
                    AX.X, ALU.add)

                # state update: c' = f*c + rs * L'; h' = o * tanh(c')
                hc1v = HC1[:].rearrange("p (t x) -> p t x", t=2)
                fov = fo[:].rearrange("p (t x) -> p t x", t=2)
                fc = wk.tile([128, NT * H], F32, tag="fc")
                nc.vector.tensor_tensor(
                    fc[:].rearrange("p (t h) -> p t h", t=2),
                    fov[:, :, 0:64], hc1v[:, :, 64:128], ALU.mult)
                for tau in range(NT):
                    nc.vector.scalar_tensor_tensor(
                        HC1[:, tau * 128 + 64:tau * 128 + 128],
                        Lp[:, tau * H:(tau + 1) * H], rs[:, tau:tau + 1],
                        fc[:, tau * H:(tau + 1) * H], ALU.mult, ALU.add)
                tc1 = wk.tile([128, NT * H], BF16, tag="tc1")
                nc.scalar.activation(
                    tc1[:].rearrange("p (t h) -> p t h", t=2),
                    hc1v[:, :, 64:128], ACTF.Tanh)
                nc.vector.tensor_tensor(
                    hc1v[:, :, 0:64],
                    fov[:, :, 64:128],
                    tc1[:].rearrange("p (t h) -> p t h", t=2), ALU.mult)

            # epilogue: capture h of the last step
            misc = mp.tile([128, 512], F32, tag="misc")
            for tau in range(NT):
                nc.tensor.transpose(
                    misc[0:128, tau * 128:(tau + 1) * 128],
                    HC1[:, tau * 128:(tau + 1) * 128], IDF[:])
            nc.vector.tensor_copy(HST[:, (n_steps - 1) * BL:n_steps * BL],
                                  misc[0:64, 0:256])

            # ================= phase 2: standard LSTM + head =================
            for t in range(n_steps):
                m2 = mp.tile([128, 512], F32, tag="misc")
                for tau in range(NT):
                    nc.tensor.transpose(
                        m2[0:64, 256 + tau * 128:256 + (tau + 1) * 128],
                        HC2[:, tau * 128:tau * 128 + 64], IDF[:])
                nc.vector.tensor_copy(H2T[:], m2[0:64, 256:512])

                g2 = gp.tile([128, 2560], F32, tag="gates")
                for tau in range(NT):
                    o0 = tau * 256
                    nc.tensor.matmul(g2[:, o0:o0 + 256], ON1[:, 0:128],
                                     B2[:], start=True, stop=False)
                    nc.tensor.matmul(g2[:, o0:o0 + 256],
                                     HST[:, t * BL + tau * 128:t * BL + (tau + 1) * 128],
                                     WCA[:], start=False, stop=False)
                    nc.tensor.matmul(g2[:, o0:o0 + 256],
                                     H2T[:, tau * 128:(tau + 1) * 128],
                                     WCB[:], start=False, stop=True)

                g2v = g2[:, 0:512].rearrange("p (t c) -> p t c", t=2)
                s2 = wk.tile([128, NT * 192], BF16, tag="s2")
                g2t = wk.tile([128, NT * H], BF16, tag="g2t")
                nc.scalar.activation(
                    s2[:].rearrange("p (t c) -> p t c", t=2),
                    g2v[:, :, 0:192], ACTF.Sigmoid)
                nc.scalar.activation(
                    g2t[:].rearrange("p (t c) -> p t c", t=2),
                    g2v[:, :, 192:256], ACTF.Tanh)

                s2v = s2[:].rearrange("p (t c) -> p t c", t=2)
                hc2v = HC2[:].rearrange("p (t x) -> p t x", t=2)
                ig = wk.tile([128, NT * H], F32, tag="ig")
                nc.vector.tensor_tensor(
                    ig[:].rearrange("p (t h) -> p t h", t=2),
                    s2v[:, :, 0:64], g2t[:].rearrange("p (t h) -> p t h", t=2),
                    ALU.mult)
                fc2 = wk.tile([128, NT * H], F32, tag="fc2")
                nc.vector.tensor_tensor(
                    fc2[:].rearrange("p (t h) -> p t h", t=2),
                    s2v[:, :, 64:128], hc2v[:, :, 64:128], ALU.mult)
                nc.vector.tensor_tensor(
                    hc2v[:, :, 64:128],
                    ig[:].rearrange("p (t h) -> p t h", t=2),
                    fc2[:].rearrange("p (t h) -> p t h", t=2), ALU.add)
                tc2 = wk.tile([128, NT * H], BF16, tag="tc2")
                nc.scalar.activation(
                    tc2[:].rearrange("p (t h) -> p t h", t=2),
                    hc2v[:, :, 64:128], ACTF.Tanh)
                nc.vector.tensor_tensor(
                    hc2v[:, :, 0:64],
                    s2v[:, :, 128:192],
                    tc2[:].rearrange("p (t h) -> p t h", t=2), ALU.mult)
                # out_t = sum_h relu(h2) * lin_w
                zz = wk.tile([128, H], F32, tag="zz")
                for tau in range(NT):
                    nc.vector.scalar_tensor_tensor(
                        zz[:], HC2[:, tau * 128:tau * 128 + 64], 0.0, LW[:],
                        ALU.max, ALU.mult,
                        accum_out=OACC[:, tau * n_steps + t:tau * n_steps + t + 1])

            ov = out.rearrange("s (tau p) o -> tau p (s o)", p=128)
            for tau in range(NT):
                nc.sync.dma_start(
                    ov[tau], OACC[:, tau * n_steps:(tau + 1) * n_steps])

    nc.finalize()
    return nc


def _prep_weights(inp):
    f32 = np.float32
    W_main, U_main, b_main = (np.asarray(inp["W_main"], f32),
                              np.asarray(inp["U_main"], f32),
                              np.asarray(inp["b_main"], f32))
    W_aux, U_aux, b_aux = (np.asarray(inp["W_aux"], f32),
                           np.asarray(inp["U_aux"], f32),
                           np.asarray(inp["b_aux"], f32))
    wall = np.zeros((KIN, 1280), f32)
    for k in range(K):
        c = 64 * k
        wall[5 + 5 * k:10 + 5 * k, c:c + 64] = W_aux[k, :, 0:64]
        wall[64:128, c:c + 64] = U_aux[k, :, 0:64]
        wall[45, c:c + 64] = b_aux[k, 0:64]
        wall[5 + 5 * k:10 + 5 * k, 512 + c:512 + c + 64] = W_aux[k, :, 64:128]
        wall[64:128, 512 + c:512 + c + 64] = U_aux[k, :, 64:128]
        wall[45, 512 + c:512 + c + 64] = b_aux[k, 64:128]
    wall[0:5, 1024:1280] = W_main
    wall[64:128, 1024:1280] = U_main
    wall[45, 1024:1280] = b_main

    watt = np.asarray(inp["W_att"], f32).T.copy()          # (64,64): rhs for v
    # phase 2: cols reordered [i f o | g] (torch gate order i,f,g,o)
    perm = np.concatenate([np.arange(0, 128), np.arange(192, 256),
                           np.arange(128, 192)])
    wc_a = np.asarray(inp["W_ih"], f32).T[:, perm].copy()
    wc_b = np.asarray(inp["W_hh"], f32).T[:, perm].copy()
    bias2 = (np.asarray(inp["b_ih"], f32) + np.asarray(inp["b_hh"], f32))[perm][None, :]
    linw = np.broadcast_to(np.asarray(inp["lin_W"], f32), (128, H)).copy()

    bf = ml_dtypes.bfloat16
    return dict(
        wall=wall.astype(bf), watt=watt.astype(bf),
        wc_a=wc_a.astype(bf), wc_b=wc_b.astype(bf), bias2=bias2.astype(bf),
        linw=linw.astype(bf), ones1=np.ones((1, 128), bf),
        onesrow=np.ones((1, BL), bf),
        idf32=np.eye(128, dtype=f32),
    )


def kernel(**inputs) -> np.ndarray:
    n_steps = int(os.environ.get("KERNEL_STEPS", S))
    names = ["Y"] + ["x%d" % i for i in range(1, 9)]
    big = np.stack([np.asarray(inputs[n], np.float32)[:n_steps] for n in names],
                   axis=1)  # (n_steps, 9, B, F)
    wmaps = _prep_weights(inputs)
    b_att = float(np.asarray(inputs["b_att"]).reshape(-1)[0])
    lin_b = float(np.asarray(inputs["lin_b"]).reshape(-1)[0])

    nc = _build(n_steps, b_att)
    in_maps = []
    for c in range(NC):
        m = dict(wmaps)
        m["xin"] = np.ascontiguousarray(big[:, :, c * BL:(c + 1) * BL, :])
        in_maps.append(m)

    trace = bool(int(os.environ.get("KERNEL_TRACE", "0")))
    res = run_bass_kernel_spmd(nc, in_maps, core_ids=list(range(NC)),
                               trace=trace)
    LAST_RESULTS["exec_time_ns"] = res.exec_time_ns
    LAST_RESULTS["trace"] = res.instructions_and_trace

    outs = [r["out"] for r in res.results]  # each (n_steps, BL, 1)
    full = np.concatenate(outs, axis=1) + lin_b
    return full.astype(np.float32)

